# revision 13
# baseline (speedup 1.0000x reference)
"""Trainium2 Bass kernel for nn_DecoderLSTM (30-step decoder LSTM, npeds=8192,
hidden=256, embed=64), data-parallel over peds across 8 NeuronCores.

Layout strategy (per core, 1024 peds split into 2 pipelined halves of 512):
  - Everything "transposed": partitions = feature dims, free = peds.
  - Gates computed as gatesT = Wstk.T-slices @ [h; dec; ones] with gate rows
    pre-permuted so each hidden chunk's (i,f,o) land contiguous in one PSUM
    tile (single fused sigmoid) and g separately (tanh).
  - Bias folded into the K=65 input matmul via a constant ones row.
  - LayerNorm2 folded algebraically: rel = sigmoid((A@h) * rsqrt(V) + b')
    with A = (g*pos_W) - rowsum(g*pos_W)/H, V = E[h^2] - mu^2 + eps.
    Stats computed by PE matmuls (A, ones/H against h and h^2).
  - Per-ped scalar tail runs in a 32x32 block-transposed domain so each op
    is [32, 16] (cost ~ free size on DVE/ACT). rsqrt via int bit-trick seed
    + Newton (fp32, avoids ACT table switch; ACT Rsqrt is banned anyway).
  - LayerNorm1+embedding folded: ln1(p) = (s, -s) with s = e*rsqrt(e^2+4eps),
    e = p0-p1; dec_in = prelu(s*w_emb + emb_b', 0.01) via one outer-product
    matmul + one Prelu activation.
  - last_pos / lp carry is dead code (never affects output) -> dropped.

The only ACT functions used are Sigmoid/Tanh/Prelu (+Copy), all in one
activation table set -> single table load for the whole kernel.
"""
import os
import sys

for _p in ("/root/.axon_site/_ro/trn_rl_repo", "/opt/trn_rl_repo"):
    if os.path.isdir(_p) and _p not in sys.path:
        sys.path.insert(0, _p)

import numpy as np
import ml_dtypes

import concourse.bass as bass
import concourse.tile as tile
from concourse import bacc, mybir
from concourse import bass_utils
from concourse.bass_interp import get_hw_module


def _ensure_ntff_hook_module():
    """Provide antenv.axon_hooks if the image ships without it, so
    run_bass_kernel_spmd(trace=True) can capture NTFF profiles."""
    try:
        from antenv import axon_hooks  # noqa: F401
        return
    except ImportError:
        pass
    import types

    mod = types.ModuleType("antenv.axon_hooks")
    mod._HOOK = None

    def set_axon_ntff_profile_hook(hook):
        mod._HOOK = hook

    def get_axon_ntff_profile_hook():
        if mod._HOOK is None:
            try:
                from trn_agent_boot.trn_boot import _ntff_profile_via_ctypes
                mod._HOOK = _ntff_profile_via_ctypes("/opt/axon/libaxon_pjrt.so")
            except Exception:
                mod._HOOK = None
        return mod._HOOK

    mod.set_axon_ntff_profile_hook = set_axon_ntff_profile_hook
    mod.get_axon_ntff_profile_hook = get_axon_ntff_profile_hook
    sys.modules["antenv.axon_hooks"] = mod
    try:
        import antenv
        antenv.axon_hooks = mod
    except ImportError:
        pass


_ensure_ntff_hook_module()

F32 = mybir.dt.float32
BF16 = mybir.dt.bfloat16
DT = BF16          # dtype for matmul operands / states / gate elementwise
I32 = mybir.dt.int32
AF = mybir.ActivationFunctionType
OP = mybir.AluOpType

N_CORES = 8
NPEDS = 8192
NP_CORE = NPEDS // N_CORES      # 1024
HALF = NP_CORE // 2             # 512
H = 256
E = 64
T = 30
EPS = 1e-5
LEAK = 0.01
MAGIC = 0x5F3759DF

# number of Newton iterations for the two rsqrt sites
NR_R = 1   # LayerNorm2 rsqrt
NR_S = 1   # ln1/embed rsqrt (very low downstream sensitivity)


def _build_program():
    nc = bacc.Bacc(
        "TRN2",
        target_bir_lowering=False,
        debug=False,
        enable_asserts=False,
        num_devices=N_CORES,
    )

    # ---- DRAM I/O ----
    d = {}
    d["LW1"] = nc.dram_tensor("LW1", [128, 1024], DT, kind="ExternalInput")
    d["LW2"] = nc.dram_tensor("LW2", [128, 1024], DT, kind="ExternalInput")
    d["LW0"] = nc.dram_tensor("LW0", [128, 1024], DT, kind="ExternalInput")
    d["AS"] = nc.dram_tensor("AS", [128, 64], DT, kind="ExternalInput")
    d["SQ"] = nc.dram_tensor("SQ", [128, 64], DT, kind="ExternalInput")
    d["GB"] = nc.dram_tensor("GB", [128, 8], F32, kind="ExternalInput")
    d["WEMB"] = nc.dram_tensor("WEMB", [1, 128], DT, kind="ExternalInput")
    d["EMBB"] = nc.dram_tensor("EMBB", [128, 1], F32, kind="ExternalInput")
    d["PB0"] = nc.dram_tensor("PB0", [32, 1], F32, kind="ExternalInput")
    d["PB1"] = nc.dram_tensor("PB1", [32, 1], F32, kind="ExternalInput")
    d["DEC0"] = nc.dram_tensor("DEC0", [128, NP_CORE], DT, kind="ExternalInput")
    d["H0A"] = nc.dram_tensor("H0A", [128, NP_CORE], DT, kind="ExternalInput")
    d["H0B"] = nc.dram_tensor("H0B", [128, NP_CORE], DT, kind="ExternalInput")
    d["C0A"] = nc.dram_tensor("C0A", [128, NP_CORE], DT, kind="ExternalInput")
    d["C0B"] = nc.dram_tensor("C0B", [128, NP_CORE], DT, kind="ExternalInput")
    out_t = nc.dram_tensor("OUT", [T, 2, NP_CORE], F32, kind="ExternalOutput")

    with tile.TileContext(nc) as tc:
        with (
            tc.tile_pool(name="weights", bufs=1) as wp,
            tc.tile_pool(name="state", bufs=1) as sp,
            tc.tile_pool(name="acts", bufs=4) as ap_,
            tc.tile_pool(name="dve", bufs=4) as dp,
            tc.tile_pool(name="tail", bufs=2) as tp,
            tc.tile_pool(name="pgate", bufs=3, space="PSUM") as pgate,
            tc.tile_pool(name="pdum", bufs=1, space="PSUM") as pdum,
            tc.tile_pool(name="pst", bufs=2, space="PSUM") as pst,
            tc.tile_pool(name="pemb", bufs=2, space="PSUM") as pemb,
        ):
            # ---- persistent weights in SBUF ----
            LW1 = wp.tile([128, 1024], DT, tag="LW1")
            LW2 = wp.tile([128, 1024], DT, tag="LW2")
            LW0 = wp.tile([128, 1024], DT, tag="LW0")
            AS = wp.tile([128, 64], DT, tag="AS")
            SQ = wp.tile([128, 64], DT, tag="SQ")
            GB = wp.tile([128, 8], F32, tag="GB")
            WEMB = wp.tile([1, 128], DT, tag="WEMB")
            EMBB = wp.tile([128, 1], F32, tag="EMBB")
            PB0 = wp.tile([32, 1], F32, tag="PB0")
            PB1 = wp.tile([32, 1], F32, tag="PB1")
            IONE = wp.tile([32, 16], I32, tag="IONE")
            IMAGIC = wp.tile([32, 16], I32, tag="IMAGIC")
            for name, tl in [("LW1", LW1), ("LW2", LW2), ("LW0", LW0),
                             ("AS", AS), ("SQ", SQ), ("GB", GB),
                             ("WEMB", WEMB), ("EMBB", EMBB),
                             ("PB0", PB0), ("PB1", PB1)]:
                nc.sync.dma_start(tl[:], d[name].ap())
            nc.vector.memset(IONE[:], 1)
            nc.vector.memset(IMAGIC[:], MAGIC)

            # ---- persistent state tiles: [half][parity] ----
            T0 = [[sp.tile([128, HALF], DT, name=f"T0_{h}_{p}",
                           tag=f"T0_{h}_{p}") for p in range(2)]
                  for h in range(2)]
            T1 = [[sp.tile([128, HALF], DT, name=f"T1_{h}_{p}",
                           tag=f"T1_{h}_{p}") for p in range(2)]
                  for h in range(2)]
            T2 = [[sp.tile([128, HALF], DT, name=f"T2_{h}_{p}",
                           tag=f"T2_{h}_{p}") for p in range(2)]
                  for h in range(2)]
            CS = [[[sp.tile([128, HALF], DT, name=f"C{ch}_{h}_{p}",
                            tag=f"C{ch}_{h}_{p}")
                    for p in range(2)] for h in range(2)] for ch in range(2)]
            TLS = [sp.tile([32, HALF], F32, name=f"tailS_{h}",
                           tag=f"tailS_{h}") for h in range(2)]
            SSB = [sp.tile([32, HALF], DT, name=f"sS_{h}",
                           tag=f"sS_{h}") for h in range(2)]
            for h in range(2):
                nc.vector.memset(TLS[h][:], 0.0)
                nc.vector.memset(SSB[h][:], 0.0)

            for h in range(2):
                cols = slice(h * HALF, (h + 1) * HALF)
                nc.sync.dma_start(T0[h][0][:], d["DEC0"].ap()[:, cols])
                nc.sync.dma_start(T1[h][0][:], d["H0A"].ap()[:, cols])
                nc.sync.dma_start(T2[h][0][:], d["H0B"].ap()[:, cols])
                nc.sync.dma_start(CS[0][h][0][:], d["C0A"].ap()[:, cols])
                nc.sync.dma_start(CS[1][h][0][:], d["C0B"].ap()[:, cols])

            def rsqrt_tail(v_tile, n_newton, tag):
                """v_tile: [32,16] fp32 (contiguous). returns tile with rsqrt."""
                y = dp.tile([32, 16], F32, tag=f"y_{tag}")
                vi = v_tile.bitcast(I32)
                yi = y.bitcast(I32)
                # y = bits(MAGIC - (v >> 1))
                sh = dp.tile([32, 16], I32, tag=f"sh_{tag}")
                nc.vector.tensor_tensor(sh[:], vi[:], IONE[:], OP.arith_shift_right)
                nc.vector.tensor_tensor(yi[:], IMAGIC[:], sh[:], OP.subtract)
                for it in range(n_newton):
                    a = dp.tile([32, 16], F32, tag=f"nra_{tag}_{it}")
                    nc.vector.tensor_tensor(a[:], y[:], y[:], OP.mult)
                    nc.vector.scalar_tensor_tensor(a[:], a[:], -0.5, v_tile[:],
                                                   OP.mult, OP.mult)
                    nc.vector.scalar_tensor_tensor(y[:], a[:], 1.5, y[:],
                                                   OP.add, OP.mult)
                return y

            # ---- time loop ----
            # Emission order per step is engine-FIFO aware: all gate work for
            # both halves first, then stats, then tails, then embeds -- so no
            # instruction head-of-line-blocks independent later work.
            USE_TANH_S = True
            TANH_S_SCALE = 0.88 / float(np.sqrt(4.0 * EPS))

            def gates_and_cell(h, p, q):
                hsq = []
                for ch in range(2):
                    acts = {}
                    for gname in ("f", "i", "g", "o"):
                        j = {"i": 0, "f": 1, "o": 2, "g": 3}[gname]
                        m = (ch * 4 + j) * 128
                        gp_t = pgate.tile([128, 512], F32,
                                          name=f"gp_{gname}", tag="gate")
                        nc.tensor.matmul(gp_t[:], LW1[:, m:m + 128],
                                         T1[h][p][:], start=True, stop=False)
                        nc.tensor.matmul(gp_t[:], LW2[:, m:m + 128],
                                         T2[h][p][:], start=False, stop=False)
                        nc.tensor.matmul(gp_t[:], LW0[0:64, m:m + 128],
                                         T0[h][p][0:64, :], start=False,
                                         stop=True)
                        a = ap_.tile([128, 512], DT, name=f"a_{gname}",
                                     tag=f"a_{gname}")
                        col = ch * 4 + j
                        nc.scalar.activation(
                            a[:], gp_t[:],
                            AF.Tanh if gname == "g" else AF.Sigmoid,
                            bias=GB[:, col:col + 1])
                        acts[gname] = a
                    m1 = dp.tile([128, 512], DT, tag="m1")
                    nc.vector.tensor_tensor(m1[:], acts["f"][:], CS[ch][h][p][:],
                                            OP.mult)
                    m2 = dp.tile([128, 512], DT, tag="m2")
                    nc.vector.tensor_tensor(m2[:], acts["i"][:], acts["g"][:],
                                            OP.mult)
                    cn = CS[ch][h][q]
                    nc.vector.tensor_tensor(cn[:], m1[:], m2[:], OP.add)
                    tc_ = ap_.tile([128, 512], DT, tag="tc")
                    nc.scalar.activation(tc_[:], cn[:], AF.Tanh)
                    hn = (T1 if ch == 0 else T2)[h][q]
                    nc.vector.tensor_tensor(hn[:], acts["o"][:], tc_[:], OP.mult)
                    hs = dp.tile([128, 512], DT, tag=f"hsq{ch}")
                    nc.gpsimd.tensor_tensor(hs[:], hn[:], hn[:], OP.mult)
                    hsq.append(hs)
                return hsq

            def stats(h, q, hsq):
                st = pst.tile([32, 512], F32, tag="st")
                nc.tensor.matmul(st[:], AS[:, 0:32], T1[h][q][:],
                                 start=True, stop=False)
                nc.tensor.matmul(st[:], AS[:, 32:64], T2[h][q][:],
                                 start=False, stop=False)
                nc.tensor.matmul(st[:], SQ[:, 0:32], hsq[0][:],
                                 start=False, stop=False)
                nc.tensor.matmul(st[:], SQ[:, 32:64], hsq[1][:],
                                 start=False, stop=True)
                return st

            def tail(t, h, st):
                tailT = tp.tile([32, 512], F32, tag="tailT")
                nc.vector.transpose(tailT[:], st[:])
                c_num0 = tailT[:, 0::32]
                c_num1 = tailT[:, 1::32]
                c_mu = tailT[:, 2::32]
                c_eh2 = tailT[:, 3::32]

                mu2 = dp.tile([32, 16], F32, tag="mu2")
                nc.vector.scalar_tensor_tensor(mu2[:], c_mu, -1.0, c_mu,
                                               OP.mult, OP.mult)
                V = dp.tile([32, 16], F32, tag="V")
                nc.vector.scalar_tensor_tensor(V[:], mu2[:], EPS, c_eh2,
                                               OP.add, OP.add)
                r = rsqrt_tail(V, NR_R, "r")

                tailS = TLS[h]
                z0 = dp.tile([32, 16], F32, tag="z0")
                nc.vector.tensor_tensor(z0[:], c_num0, r[:], OP.mult)
                z1 = dp.tile([32, 16], F32, tag="z1")
                nc.vector.tensor_tensor(z1[:], c_num1, r[:], OP.mult)
                nc.scalar.activation(tailS[:, 1::32], z0[:], AF.Sigmoid,
                                     bias=PB0[:])
                nc.scalar.activation(tailS[:, 2::32], z1[:], AF.Sigmoid,
                                     bias=PB1[:])
                e = dp.tile([32, 16], F32, tag="e")
                nc.vector.tensor_tensor(e[:], tailS[:, 1::32],
                                        tailS[:, 2::32], OP.subtract)
                # keep-warm: trivial matmuls dependent on mid-tail data keep
                # the PE HAM activity window busy during the tail bubble
                dmy = pdum.tile([1, 64], F32, tag="dmy")
                nc.tensor.matmul(dmy[:], tailT[0:1, 0:1], tailT[0:1, 0:64],
                                 start=True, stop=True)
                sS = SSB[h]
                if USE_TANH_S:
                    nc.scalar.activation(sS[:, 0::32], e[:], AF.Tanh,
                                         scale=TANH_S_SCALE)
                else:
                    e2 = dp.tile([32, 16], F32, tag="e2")
                    nc.vector.tensor_tensor(e2[:], e[:], e[:], OP.mult)
                    nc.vector.tensor_scalar(e2[:], e2[:], 4.0 * EPS, None,
                                            OP.add)
                    rs = rsqrt_tail(e2, NR_S, "s")
                    nc.vector.tensor_tensor(sS[:, 0::32], e[:], rs[:], OP.mult)
                dmy2 = pdum.tile([1, 64], F32, tag="dmy")
                nc.tensor.matmul(dmy2[:], tailS[0:1, 0:1], tailS[0:1, 0:64],
                                 start=True, stop=True)
                # s row (bf16) for the embed matmul -- on the critical path
                sB = tp.tile([32, 512], DT, tag="sB")
                nc.vector.transpose(sB[:], sS[:])
                # rel rows -> DRAM (off the critical path)
                tailB = tp.tile([32, 512], F32, tag="tailB")
                nc.vector.transpose(tailB[:], tailS[:])
                dst = out_t.ap()[t]
                nc.sync.dma_start(dst[:, h * HALF:(h + 1) * HALF],
                                  tailB[1:3, :])
                return sB

            def embed(h, q, sB):
                pe = pemb.tile([128, 512], F32, tag="pe")
                nc.tensor.matmul(pe[:], WEMB[:], sB[0:1, :],
                                 start=True, stop=True)
                nc.scalar.activation(T0[h][q][:], pe[:],
                                     AF.Prelu, bias=EMBB[:], alpha=LEAK)

            for t in range(T):
                p, q = t % 2, (t + 1) % 2
                hsq0 = gates_and_cell(0, p, q)
                hsq1 = gates_and_cell(1, p, q)
                st0 = stats(0, q, hsq0)
                st1 = stats(1, q, hsq1)
                sB0 = tail(t, 0, st0)
                sB1 = tail(t, 1, st1)
                embed(0, q, sB0)
                embed(1, q, sB1)

    nc.compile()
    return nc


_NC_CACHE = None


def _get_program():
    global _NC_CACHE
    if _NC_CACHE is None:
        _NC_CACHE = _build_program()
    return _NC_CACHE


def _prepare_in_maps(inputs):
    f32 = np.float32
    inp = {k: np.asarray(v, f32) for k, v in inputs.items()}
    W_ih, W_hh = inp["W_ih"], inp["W_hh"]
    bias = (inp["b_ih"] + inp["b_hh"]).astype(f32)

    # gate-row permutation: per chunk [i, f, o, g]
    perm = []
    for chunk in range(2):
        for base in (0, H, 3 * H, 2 * H):          # i, f, o, g
            start = base + chunk * 128
            perm.extend(range(start, start + 128))
    perm = np.array(perm)
    Wih_p, Whh_p, bias_p = W_ih[perm], W_hh[perm], bias[perm]

    LW1 = np.ascontiguousarray(Whh_p[:, 0:128].T)          # [128, 1024]
    LW2 = np.ascontiguousarray(Whh_p[:, 128:256].T)        # [128, 1024]
    LW0 = np.concatenate([Wih_p.T, Wih_p.T], 0)           # [128, 1024] dup rows

    emb_W, emb_b = inp["emb_W"], inp["emb_b"]
    g1, b1 = inp["ln1_g"], inp["ln1_b"]
    w_emb = (g1[0] * emb_W[:, 0] - g1[1] * emb_W[:, 1]).astype(f32)
    emb_bp = (emb_b + b1[0] * emb_W[:, 0] + b1[1] * emb_W[:, 1]).astype(f32)

    pos_W, pos_b = inp["pos_W"], inp["pos_b"]
    g2, b2 = inp["ln2_g"], inp["ln2_b"]
    posWp = (pos_W * g2[None, :]).astype(f32)
    pos_bp = (pos_b + b2 @ pos_W.T).astype(f32)
    w1 = posWp.sum(1)
    A = posWp - w1[:, None] / H                            # [2, 256]

    AS = np.zeros((128, 64), f32)
    AS[:, 0], AS[:, 1], AS[:, 2] = A[0, 0:128], A[1, 0:128], 1.0 / H
    AS[:, 32], AS[:, 33], AS[:, 34] = A[0, 128:256], A[1, 128:256], 1.0 / H
    SQ = np.zeros((128, 64), f32)
    SQ[:, 3] = 1.0 / H
    SQ[:, 35] = 1.0 / H
    GBm = np.ascontiguousarray(bias_p.reshape(8, 128).T)   # [128, 8]

    lpr = inp["last_pos_rel"]
    e0 = lpr[:, 0] - lpr[:, 1]
    s0 = e0 / np.sqrt(e0 * e0 + 4 * EPS)
    z = s0[:, None] * w_emb[None, :] + emb_bp[None, :]
    dec0 = np.where(z > 0, z, LEAK * z).astype(f32)        # [N, 64]
    dec0T = np.ascontiguousarray(dec0.T)                   # [64, N]
    dec0T = np.concatenate([dec0T, dec0T], 0)              # [128, N] dup

    h0T = np.ascontiguousarray(inp["h0"][0].T)             # [256, N]
    c0T = np.ascontiguousarray(inp["c0"][0].T)

    bf = ml_dtypes.bfloat16 if DT == BF16 else f32
    rep = {
        "LW1": LW1.astype(bf), "LW2": LW2.astype(bf),
        "LW0": np.ascontiguousarray(LW0).astype(bf),
        "AS": AS.astype(bf), "SQ": SQ.astype(bf), "GB": GBm,
        "WEMB": np.ascontiguousarray(np.concatenate([w_emb, w_emb])[None, :]).astype(bf),
        "EMBB": np.ascontiguousarray(np.concatenate([emb_bp, emb_bp])[:, None]),
        "PB0": np.full((32, 1), pos_bp[0], f32),
        "PB1": np.full((32, 1), pos_bp[1], f32),
    }
    in_maps = []
    for c in range(N_CORES):
        cols = slice(c * NP_CORE, (c + 1) * NP_CORE)
        m = dict(rep)
        m["DEC0"] = np.ascontiguousarray(dec0T[:, cols]).astype(bf)
        m["H0A"] = np.ascontiguousarray(h0T[0:128, cols]).astype(bf)
        m["H0B"] = np.ascontiguousarray(h0T[128:256, cols]).astype(bf)
        m["C0A"] = np.ascontiguousarray(c0T[0:128, cols]).astype(bf)
        m["C0B"] = np.ascontiguousarray(c0T[128:256, cols]).astype(bf)
        in_maps.append(m)
    return in_maps


def run_on_hw(inputs, trace=False, **kwargs):
    nc = _get_program()
    in_maps = _prepare_in_maps(inputs)
    old_m = nc.m
    nc.m = get_hw_module(nc.m)
    try:
        res = bass_utils.run_bass_kernel_spmd(
            nc, in_maps, core_ids=list(range(N_CORES)), trace=trace, **kwargs)
    finally:
        nc.m = old_m
    out = np.concatenate([r["OUT"] for r in res.results], axis=2)
    out = np.ascontiguousarray(out.transpose(0, 2, 1))
    return out.astype(np.float32), res


def kernel(**inputs) -> np.ndarray:
    out, _ = run_on_hw(inputs, trace=False)
    return out


# revision 14
# speedup vs baseline: 1.2280x; 1.2280x over previous
"""Trainium2 Bass kernel for nn_DecoderLSTM (30-step decoder LSTM, npeds=8192,
hidden=256, embed=64), data-parallel over peds across 8 NeuronCores.

Layout strategy (per core, 1024 peds split into 2 pipelined halves of 512):
  - Everything "transposed": partitions = feature dims, free = peds.
  - Gates computed as gatesT = Wstk.T-slices @ [h; dec; ones] with gate rows
    pre-permuted so each hidden chunk's (i,f,o) land contiguous in one PSUM
    tile (single fused sigmoid) and g separately (tanh).
  - Bias folded into the K=65 input matmul via a constant ones row.
  - LayerNorm2 folded algebraically: rel = sigmoid((A@h) * rsqrt(V) + b')
    with A = (g*pos_W) - rowsum(g*pos_W)/H, V = E[h^2] - mu^2 + eps.
    Stats computed by PE matmuls (A, ones/H against h and h^2).
  - Per-ped scalar tail runs in a 32x32 block-transposed domain so each op
    is [32, 16] (cost ~ free size on DVE/ACT). rsqrt via int bit-trick seed
    + Newton (fp32, avoids ACT table switch; ACT Rsqrt is banned anyway).
  - LayerNorm1+embedding folded: ln1(p) = (s, -s) with s = e*rsqrt(e^2+4eps),
    e = p0-p1; dec_in = prelu(s*w_emb + emb_b', 0.01) via one outer-product
    matmul + one Prelu activation.
  - last_pos / lp carry is dead code (never affects output) -> dropped.

The only ACT functions used are Sigmoid/Tanh/Prelu (+Copy), all in one
activation table set -> single table load for the whole kernel.
"""
import os
import sys

for _p in ("/root/.axon_site/_ro/trn_rl_repo", "/opt/trn_rl_repo"):
    if os.path.isdir(_p) and _p not in sys.path:
        sys.path.insert(0, _p)

import numpy as np
import ml_dtypes

import concourse.bass as bass
import concourse.tile as tile
from concourse import bacc, mybir
from concourse import bass_utils
from concourse.bass_interp import get_hw_module


def _ensure_ntff_hook_module():
    """Provide antenv.axon_hooks if the image ships without it, so
    run_bass_kernel_spmd(trace=True) can capture NTFF profiles."""
    try:
        from antenv import axon_hooks  # noqa: F401
        return
    except ImportError:
        pass
    import types

    mod = types.ModuleType("antenv.axon_hooks")
    mod._HOOK = None

    def set_axon_ntff_profile_hook(hook):
        mod._HOOK = hook

    def get_axon_ntff_profile_hook():
        if mod._HOOK is None:
            try:
                from trn_agent_boot.trn_boot import _ntff_profile_via_ctypes
                mod._HOOK = _ntff_profile_via_ctypes("/opt/axon/libaxon_pjrt.so")
            except Exception:
                mod._HOOK = None
        return mod._HOOK

    mod.set_axon_ntff_profile_hook = set_axon_ntff_profile_hook
    mod.get_axon_ntff_profile_hook = get_axon_ntff_profile_hook
    sys.modules["antenv.axon_hooks"] = mod
    try:
        import antenv
        antenv.axon_hooks = mod
    except ImportError:
        pass


_ensure_ntff_hook_module()

F32 = mybir.dt.float32
BF16 = mybir.dt.bfloat16
DT = BF16          # dtype for matmul operands / states / gate elementwise
I32 = mybir.dt.int32
AF = mybir.ActivationFunctionType
OP = mybir.AluOpType

N_CORES = 8
NPEDS = 8192
NP_CORE = NPEDS // N_CORES      # 1024
HALF = NP_CORE // 2             # 512
H = 256
E = 64
T = 30
EPS = 1e-5
LEAK = 0.01
MAGIC = 0x5F3759DF

# number of Newton iterations for the two rsqrt sites
NR_R = 1   # LayerNorm2 rsqrt
NR_S = 1   # ln1/embed rsqrt (very low downstream sensitivity)


def _build_program():
    nc = bacc.Bacc(
        "TRN2",
        target_bir_lowering=False,
        debug=False,
        enable_asserts=False,
        num_devices=N_CORES,
    )

    # ---- DRAM I/O ----
    d = {}
    d["LW1"] = nc.dram_tensor("LW1", [128, 1024], DT, kind="ExternalInput")
    d["LW2"] = nc.dram_tensor("LW2", [128, 1024], DT, kind="ExternalInput")
    d["LW0"] = nc.dram_tensor("LW0", [128, 1024], DT, kind="ExternalInput")
    d["AS"] = nc.dram_tensor("AS", [128, 64], DT, kind="ExternalInput")
    d["SQ"] = nc.dram_tensor("SQ", [128, 64], DT, kind="ExternalInput")
    d["GB"] = nc.dram_tensor("GB", [128, 8], F32, kind="ExternalInput")
    d["WEMB"] = nc.dram_tensor("WEMB", [1, 128], DT, kind="ExternalInput")
    d["EMBB"] = nc.dram_tensor("EMBB", [128, 1], F32, kind="ExternalInput")
    d["PB0"] = nc.dram_tensor("PB0", [32, 1], F32, kind="ExternalInput")
    d["PB1"] = nc.dram_tensor("PB1", [32, 1], F32, kind="ExternalInput")
    d["DEC0"] = nc.dram_tensor("DEC0", [128, NP_CORE], DT, kind="ExternalInput")
    d["H0A"] = nc.dram_tensor("H0A", [128, NP_CORE], DT, kind="ExternalInput")
    d["H0B"] = nc.dram_tensor("H0B", [128, NP_CORE], DT, kind="ExternalInput")
    d["C0A"] = nc.dram_tensor("C0A", [128, NP_CORE], DT, kind="ExternalInput")
    d["C0B"] = nc.dram_tensor("C0B", [128, NP_CORE], DT, kind="ExternalInput")
    out_t = nc.dram_tensor("OUT", [T, 2, NP_CORE], F32, kind="ExternalOutput")

    with tile.TileContext(nc) as tc:
        with (
            tc.tile_pool(name="weights", bufs=1) as wp,
            tc.tile_pool(name="state", bufs=1) as sp,
            tc.tile_pool(name="acts", bufs=4) as ap_,
            tc.tile_pool(name="dve", bufs=4) as dp,
            tc.tile_pool(name="tail", bufs=2) as tp,
            tc.tile_pool(name="pgate", bufs=4, space="PSUM") as pgate,
            tc.tile_pool(name="pst", bufs=2, space="PSUM") as pst,
            tc.tile_pool(name="pemb", bufs=2, space="PSUM") as pemb,
        ):
            # ---- persistent weights in SBUF ----
            LW1 = wp.tile([128, 1024], DT, tag="LW1")
            LW2 = wp.tile([128, 1024], DT, tag="LW2")
            LW0 = wp.tile([128, 1024], DT, tag="LW0")
            AS = wp.tile([128, 64], DT, tag="AS")
            SQ = wp.tile([128, 64], DT, tag="SQ")
            GB = wp.tile([128, 8], F32, tag="GB")
            WEMB = wp.tile([1, 128], DT, tag="WEMB")
            EMBB = wp.tile([128, 1], F32, tag="EMBB")
            PB0 = wp.tile([32, 1], F32, tag="PB0")
            PB1 = wp.tile([32, 1], F32, tag="PB1")
            IONE = wp.tile([32, 16], I32, tag="IONE")
            IMAGIC = wp.tile([32, 16], I32, tag="IMAGIC")
            for name, tl in [("LW1", LW1), ("LW2", LW2), ("LW0", LW0),
                             ("AS", AS), ("SQ", SQ), ("GB", GB),
                             ("WEMB", WEMB), ("EMBB", EMBB),
                             ("PB0", PB0), ("PB1", PB1)]:
                nc.sync.dma_start(tl[:], d[name].ap())
            nc.vector.memset(IONE[:], 1)
            nc.vector.memset(IMAGIC[:], MAGIC)

            # ---- persistent state tiles: [half][parity] ----
            T0 = [[sp.tile([128, HALF], DT, name=f"T0_{h}_{p}",
                           tag=f"T0_{h}_{p}") for p in range(2)]
                  for h in range(2)]
            T1 = [[sp.tile([128, HALF], DT, name=f"T1_{h}_{p}",
                           tag=f"T1_{h}_{p}") for p in range(2)]
                  for h in range(2)]
            T2 = [[sp.tile([128, HALF], DT, name=f"T2_{h}_{p}",
                           tag=f"T2_{h}_{p}") for p in range(2)]
                  for h in range(2)]
            CS = [[[sp.tile([128, HALF], DT, name=f"C{ch}_{h}_{p}",
                            tag=f"C{ch}_{h}_{p}")
                    for p in range(2)] for h in range(2)] for ch in range(2)]
            TLS = [sp.tile([32, HALF], F32, name=f"tailS_{h}",
                           tag=f"tailS_{h}") for h in range(2)]
            SSB = [sp.tile([32, HALF], DT, name=f"sS_{h}",
                           tag=f"sS_{h}") for h in range(2)]
            for h in range(2):
                nc.vector.memset(TLS[h][:], 0.0)
                nc.vector.memset(SSB[h][:], 0.0)

            for h in range(2):
                cols = slice(h * HALF, (h + 1) * HALF)
                nc.sync.dma_start(T0[h][0][:], d["DEC0"].ap()[:, cols])
                nc.sync.dma_start(T1[h][0][:], d["H0A"].ap()[:, cols])
                nc.sync.dma_start(T2[h][0][:], d["H0B"].ap()[:, cols])
                nc.sync.dma_start(CS[0][h][0][:], d["C0A"].ap()[:, cols])
                nc.sync.dma_start(CS[1][h][0][:], d["C0B"].ap()[:, cols])

            def rsqrt_tail(v_tile, n_newton, tag):
                """v_tile: [32,16] fp32 (contiguous). returns tile with rsqrt."""
                y = dp.tile([32, 16], F32, tag=f"y_{tag}")
                vi = v_tile.bitcast(I32)
                yi = y.bitcast(I32)
                # y = bits(MAGIC - (v >> 1))
                sh = dp.tile([32, 16], I32, tag=f"sh_{tag}")
                nc.vector.tensor_tensor(sh[:], vi[:], IONE[:], OP.arith_shift_right)
                nc.vector.tensor_tensor(yi[:], IMAGIC[:], sh[:], OP.subtract)
                for it in range(n_newton):
                    a = dp.tile([32, 16], F32, tag=f"nra_{tag}_{it}")
                    nc.vector.tensor_tensor(a[:], y[:], y[:], OP.mult)
                    nc.vector.scalar_tensor_tensor(a[:], a[:], -0.5, v_tile[:],
                                                   OP.mult, OP.mult)
                    nc.vector.scalar_tensor_tensor(y[:], a[:], 1.5, y[:],
                                                   OP.add, OP.mult)
                return y

            # ---- time loop ----
            # Emission order per step is engine-FIFO aware: all gate work for
            # both halves first, then stats, then tails, then embeds -- so no
            # instruction head-of-line-blocks independent later work.
            USE_TANH_S = True
            TANH_S_SCALE = 0.88 / float(np.sqrt(4.0 * EPS))

            def gates_and_cell(h, p, q):
                hsq = []
                for ch in range(2):
                    acts = {}
                    for gname in ("f", "i", "g", "o"):
                        j = {"i": 0, "f": 1, "o": 2, "g": 3}[gname]
                        m = (ch * 4 + j) * 128
                        gp_t = pgate.tile([128, 512], F32,
                                          name=f"gp_{gname}", tag="gate")
                        nc.tensor.matmul(gp_t[:], LW1[:, m:m + 128],
                                         T1[h][p][:], start=True, stop=False)
                        nc.tensor.matmul(gp_t[:], LW2[:, m:m + 128],
                                         T2[h][p][:], start=False, stop=False)
                        nc.tensor.matmul(gp_t[:], LW0[0:64, m:m + 128],
                                         T0[h][p][0:64, :], start=False,
                                         stop=True)
                        a = ap_.tile([128, 512], DT, name=f"a_{gname}",
                                     tag=f"a_{gname}")
                        col = ch * 4 + j
                        nc.scalar.activation(
                            a[:], gp_t[:],
                            AF.Tanh if gname == "g" else AF.Sigmoid,
                            bias=GB[:, col:col + 1])
                        acts[gname] = a
                    m1 = dp.tile([128, 512], DT, tag="m1")
                    nc.vector.tensor_tensor(m1[:], acts["f"][:], CS[ch][h][p][:],
                                            OP.mult)
                    m2 = dp.tile([128, 512], DT, tag="m2")
                    nc.vector.tensor_tensor(m2[:], acts["i"][:], acts["g"][:],
                                            OP.mult)
                    cn = CS[ch][h][q]
                    nc.vector.tensor_tensor(cn[:], m1[:], m2[:], OP.add)
                    tc_ = ap_.tile([128, 512], DT, tag="tc")
                    nc.scalar.activation(tc_[:], cn[:], AF.Tanh)
                    hn = (T1 if ch == 0 else T2)[h][q]
                    nc.vector.tensor_tensor(hn[:], acts["o"][:], tc_[:], OP.mult)
                    hs = dp.tile([128, 512], DT, tag=f"hsq{ch}")
                    nc.gpsimd.tensor_tensor(hs[:], hn[:], hn[:], OP.mult)
                    hsq.append(hs)
                return hsq

            def stats(h, q, hsq):
                st = pst.tile([32, 512], F32, tag="st")
                nc.tensor.matmul(st[:], AS[:, 0:32], T1[h][q][:],
                                 start=True, stop=False)
                nc.tensor.matmul(st[:], AS[:, 32:64], T2[h][q][:],
                                 start=False, stop=False)
                nc.tensor.matmul(st[:], SQ[:, 0:32], hsq[0][:],
                                 start=False, stop=False)
                nc.tensor.matmul(st[:], SQ[:, 32:64], hsq[1][:],
                                 start=False, stop=True)
                return st

            def tail(t, h, st):
                tailT = tp.tile([32, 512], F32, tag="tailT")
                nc.vector.transpose(tailT[:], st[:])
                c_num0 = tailT[:, 0::32]
                c_num1 = tailT[:, 1::32]
                c_mu = tailT[:, 2::32]
                c_eh2 = tailT[:, 3::32]

                mu2 = dp.tile([32, 16], F32, tag="mu2")
                nc.vector.scalar_tensor_tensor(mu2[:], c_mu, -1.0, c_mu,
                                               OP.mult, OP.mult)
                V = dp.tile([32, 16], F32, tag="V")
                nc.vector.scalar_tensor_tensor(V[:], mu2[:], EPS, c_eh2,
                                               OP.add, OP.add)
                r = rsqrt_tail(V, NR_R, "r")

                tailS = TLS[h]
                z0 = dp.tile([32, 16], F32, tag="z0")
                nc.vector.tensor_tensor(z0[:], c_num0, r[:], OP.mult)
                z1 = dp.tile([32, 16], F32, tag="z1")
                nc.vector.tensor_tensor(z1[:], c_num1, r[:], OP.mult)
                nc.scalar.activation(tailS[:, 1::32], z0[:], AF.Sigmoid,
                                     bias=PB0[:])
                nc.scalar.activation(tailS[:, 2::32], z1[:], AF.Sigmoid,
                                     bias=PB1[:])
                e = dp.tile([32, 16], F32, tag="e")
                nc.vector.tensor_tensor(e[:], tailS[:, 1::32],
                                        tailS[:, 2::32], OP.subtract)
                sS = SSB[h]
                if USE_TANH_S:
                    nc.scalar.activation(sS[:, 0::32], e[:], AF.Tanh,
                                         scale=TANH_S_SCALE)
                else:
                    e2 = dp.tile([32, 16], F32, tag="e2")
                    nc.vector.tensor_tensor(e2[:], e[:], e[:], OP.mult)
                    nc.vector.tensor_scalar(e2[:], e2[:], 4.0 * EPS, None,
                                            OP.add)
                    rs = rsqrt_tail(e2, NR_S, "s")
                    nc.vector.tensor_tensor(sS[:, 0::32], e[:], rs[:], OP.mult)
                # s row (bf16) for the embed matmul -- on the critical path
                sB = tp.tile([32, 512], DT, tag="sB")
                nc.vector.transpose(sB[:], sS[:])
                # rel rows -> DRAM (off the critical path)
                tailB = tp.tile([32, 512], F32, tag="tailB")
                nc.vector.transpose(tailB[:], tailS[:])
                dst = out_t.ap()[t]
                nc.sync.dma_start(dst[:, h * HALF:(h + 1) * HALF],
                                  tailB[1:3, :])
                return sB

            def embed(h, q, sB):
                pe = pemb.tile([128, 512], F32, tag="pe")
                nc.tensor.matmul(pe[:], WEMB[:], sB[0:1, :],
                                 start=True, stop=True)
                nc.scalar.activation(T0[h][q][:], pe[:],
                                     AF.Prelu, bias=EMBB[:], alpha=LEAK)

            for t in range(T):
                p, q = t % 2, (t + 1) % 2
                hsq0 = gates_and_cell(0, p, q)
                hsq1 = gates_and_cell(1, p, q)
                st0 = stats(0, q, hsq0)
                st1 = stats(1, q, hsq1)
                sB0 = tail(t, 0, st0)
                sB1 = tail(t, 1, st1)
                embed(0, q, sB0)
                embed(1, q, sB1)

    nc.compile()
    return nc


_NC_CACHE = None


def _get_program():
    global _NC_CACHE
    if _NC_CACHE is None:
        _NC_CACHE = _build_program()
    return _NC_CACHE


def _prepare_in_maps(inputs):
    f32 = np.float32
    inp = {k: np.asarray(v, f32) for k, v in inputs.items()}
    W_ih, W_hh = inp["W_ih"], inp["W_hh"]
    bias = (inp["b_ih"] + inp["b_hh"]).astype(f32)

    # gate-row permutation: per chunk [i, f, o, g]
    perm = []
    for chunk in range(2):
        for base in (0, H, 3 * H, 2 * H):          # i, f, o, g
            start = base + chunk * 128
            perm.extend(range(start, start + 128))
    perm = np.array(perm)
    Wih_p, Whh_p, bias_p = W_ih[perm], W_hh[perm], bias[perm]

    LW1 = np.ascontiguousarray(Whh_p[:, 0:128].T)          # [128, 1024]
    LW2 = np.ascontiguousarray(Whh_p[:, 128:256].T)        # [128, 1024]
    LW0 = np.concatenate([Wih_p.T, Wih_p.T], 0)           # [128, 1024] dup rows

    emb_W, emb_b = inp["emb_W"], inp["emb_b"]
    g1, b1 = inp["ln1_g"], inp["ln1_b"]
    w_emb = (g1[0] * emb_W[:, 0] - g1[1] * emb_W[:, 1]).astype(f32)
    emb_bp = (emb_b + b1[0] * emb_W[:, 0] + b1[1] * emb_W[:, 1]).astype(f32)

    pos_W, pos_b = inp["pos_W"], inp["pos_b"]
    g2, b2 = inp["ln2_g"], inp["ln2_b"]
    posWp = (pos_W * g2[None, :]).astype(f32)
    pos_bp = (pos_b + b2 @ pos_W.T).astype(f32)
    w1 = posWp.sum(1)
    A = posWp - w1[:, None] / H                            # [2, 256]

    AS = np.zeros((128, 64), f32)
    AS[:, 0], AS[:, 1], AS[:, 2] = A[0, 0:128], A[1, 0:128], 1.0 / H
    AS[:, 32], AS[:, 33], AS[:, 34] = A[0, 128:256], A[1, 128:256], 1.0 / H
    SQ = np.zeros((128, 64), f32)
    SQ[:, 3] = 1.0 / H
    SQ[:, 35] = 1.0 / H
    GBm = np.ascontiguousarray(bias_p.reshape(8, 128).T)   # [128, 8]

    lpr = inp["last_pos_rel"]
    e0 = lpr[:, 0] - lpr[:, 1]
    s0 = e0 / np.sqrt(e0 * e0 + 4 * EPS)
    z = s0[:, None] * w_emb[None, :] + emb_bp[None, :]
    dec0 = np.where(z > 0, z, LEAK * z).astype(f32)        # [N, 64]
    dec0T = np.ascontiguousarray(dec0.T)                   # [64, N]
    dec0T = np.concatenate([dec0T, dec0T], 0)              # [128, N] dup

    h0T = np.ascontiguousarray(inp["h0"][0].T)             # [256, N]
    c0T = np.ascontiguousarray(inp["c0"][0].T)

    bf = ml_dtypes.bfloat16 if DT == BF16 else f32
    rep = {
        "LW1": LW1.astype(bf), "LW2": LW2.astype(bf),
        "LW0": np.ascontiguousarray(LW0).astype(bf),
        "AS": AS.astype(bf), "SQ": SQ.astype(bf), "GB": GBm,
        "WEMB": np.ascontiguousarray(np.concatenate([w_emb, w_emb])[None, :]).astype(bf),
        "EMBB": np.ascontiguousarray(np.concatenate([emb_bp, emb_bp])[:, None]),
        "PB0": np.full((32, 1), pos_bp[0], f32),
        "PB1": np.full((32, 1), pos_bp[1], f32),
    }
    in_maps = []
    for c in range(N_CORES):
        cols = slice(c * NP_CORE, (c + 1) * NP_CORE)
        m = dict(rep)
        m["DEC0"] = np.ascontiguousarray(dec0T[:, cols]).astype(bf)
        m["H0A"] = np.ascontiguousarray(h0T[0:128, cols]).astype(bf)
        m["H0B"] = np.ascontiguousarray(h0T[128:256, cols]).astype(bf)
        m["C0A"] = np.ascontiguousarray(c0T[0:128, cols]).astype(bf)
        m["C0B"] = np.ascontiguousarray(c0T[128:256, cols]).astype(bf)
        in_maps.append(m)
    return in_maps


def run_on_hw(inputs, trace=False, **kwargs):
    nc = _get_program()
    in_maps = _prepare_in_maps(inputs)
    old_m = nc.m
    nc.m = get_hw_module(nc.m)
    try:
        res = bass_utils.run_bass_kernel_spmd(
            nc, in_maps, core_ids=list(range(N_CORES)), trace=trace, **kwargs)
    finally:
        nc.m = old_m
    out = np.concatenate([r["OUT"] for r in res.results], axis=2)
    out = np.ascontiguousarray(out.transpose(0, 2, 1))
    return out.astype(np.float32), res


def kernel(**inputs) -> np.ndarray:
    out, _ = run_on_hw(inputs, trace=False)
    return out
